# revision 26
# baseline (speedup 1.0000x reference)
"""EmbeddingBag-mean (padded ragged gather + masked mean) on 8 Trainium2 cores.

Strategy (data-parallel over batch):
  - Each of the 8 cores owns B/8 = 2048 samples; the embedding table is
    replicated to every core's HBM as fp16, rows padded to 128 elements
    (256 B stride) so the MoE `dma_gather` ucode (stride in 256 B units,
    int16 indices) can gather single 128 B rows.
  - int16 indices only reach 32768 rows, so the gather runs as 4 passes
    over overlapping 32768-row windows (bases ~22412 apart).  A zero
    sentinel row sits at each window base (relative index 0).  Because the
    windows overlap, each sample distributes its indices among feasible
    passes to equalize its per-pass counts, which keeps the per-block
    per-pass slot maxima near len/4 (instead of len/4 + 3 sigma).
  - Samples are globally length-sorted and dealt to (block, core,
    partition) so each block of 128 partitions holds samples of nearly
    equal length; block b needs G[b][q] gather slots for pass q (max over
    the 8 cores' blocks; one SPMD module).  Pad slots -> sentinel row.
  - Device kernel (per core), per block of 128 samples:
      1. 4x dma_gather (one per pass, 4 SWDGE queues round-robin):
         g[p, off_q + j, :] = window_q[idx16[...], :64]   (128 B descs)
      2. one DVE tensor_reduce over all slot columns (fp16 in, fp32 out)
      3. ACT Copy-with-scale by 1/max(len,1) (per-partition scalar)
      4. DMA the [128, 64] fp32 block out
  - Host un-permutes the global deal and returns [B, 64] fp32.
"""

import numpy as np

try:
    import concourse.bacc as bacc
except ImportError:  # harness containers keep the repo at /opt/trn_rl_repo
    import sys

    sys.path.insert(0, "/opt/trn_rl_repo")
    import concourse.bacc as bacc

import concourse.bass as bass
import concourse.mybir as mybir
import concourse.tile as tile
from concourse import bass_utils

B, L, V, D = 16384, 50, 100000, 64
NCORES = 8
P = 128
BC = B // NCORES  # 2048 samples per core
NBLK = BC // P  # 16 blocks of 128 samples
NQ = 4  # gather passes (overlapping windows)
WIN = 32768  # int16-reachable rows per pass
DEVROWS = V + NQ  # table + one zero sentinel per window

# window bases in device-table row space (sentinel zero row at each base)
_SPACING = -(-(DEVROWS - WIN) // (NQ - 1))  # ceil
BASES = [q * _SPACING for q in range(NQ)]
assert BASES[-1] + WIN >= DEVROWS

_CACHE: dict = {}


def _manual_dma_gather(nc, out_ap, in_ap, idxs_ap, num_idxs, elem_size,
                       queue_num, single_packet):
    """dma_gather without the elem_size_bytes%256 assert (stride is 256B)."""
    gp = nc.gpsimd
    _in_ap = gp.lower_ap_dma(in_ap, for_custom_bir_dma=True)
    _idxs_ap = gp.lower_ap(idxs_ap)
    _out_ap = gp.lower_ap(out_ap)
    stride_bytes = in_ap.ap[0][0] * mybir.dt.size(in_ap.dtype)
    assert stride_bytes % 256 == 0
    return gp.add_instruction(
        mybir.InstDMAGatherAnt(
            name=nc.get_next_instruction_name(),
            ins=[*_in_ap, _idxs_ap, gp.lower_val_access(gp.to_reg(num_idxs))],
            outs=[_out_ap],
            transpose=False,
            num_idxs=num_idxs,
            elem_size=elem_size,
            stride_bytes_256=stride_bytes // 256,
            gen_mode=0,
            single_packet=single_packet,
            queue_num=queue_num,
            sbuf_tokens_per_rank=0,
            sbuf_free_dim_per_rank=0,
            sbuf_free_dim_pad_per_rank=0,
            sbuf_byte_offset=0,
        )
    )


def build(g_sched, reps: int = 1, mode: str = "full", qpat=None):
    """Build + compile the per-core Bass module.

    g_sched: [NBLK][NQ] gather slot counts (>=1 each).
    reps > 1 wraps the block loop in tc.For_i for slope timing.
    mode: "full" | "gather" (skip reduce/scale/out) | "nored" (skip reduce).
    """
    g_sched = [list(r) for r in g_sched]
    assert len(g_sched) == NBLK and all(len(r) == NQ for r in g_sched)
    gtot = [sum(r) for r in g_sched]
    g_max = max(gtot)
    # idx16 column layout: per (block, pass) a run of G*P/16 int16 columns
    wcols = [[g * P // 16 for g in r] for r in g_sched]
    WC = sum(sum(r) for r in wcols)

    nc = bacc.Bacc("TRN2", target_bir_lowering=False, debug=False,
                   num_swdge_queues=NQ)
    table = nc.dram_tensor("table", [DEVROWS, P], mybir.dt.float16,
                           kind="ExternalInput")
    idx = nc.dram_tensor("idx", [P, WC], mybir.dt.int16, kind="ExternalInput")
    inv_len = nc.dram_tensor("inv_len", [P, NBLK], mybir.dt.float32,
                             kind="ExternalInput")
    out = nc.dram_tensor("out", [NBLK, P, D], mybir.dt.float32,
                         kind="ExternalOutput")

    with tile.TileContext(nc) as tc:
        with (
            tc.tile_pool(name="const", bufs=1) as cpool,
            tc.tile_pool(name="gather", bufs=4) as gpool,
            tc.tile_pool(name="res", bufs=4) as rpool,
        ):
            idx_sb = cpool.tile([P, WC], mybir.dt.int16)
            nc.sync.dma_start(idx_sb[:], idx.ap())
            invl_sb = cpool.tile([P, NBLK], mybir.dt.float32)
            nc.sync.dma_start(invl_sb[:], inv_len.ap())

            # one gather per (block, pass).  Queue pattern alternates between
            # blocks so the big outer passes split evenly across queues, while
            # staying periodic in issue order (Tile's DMASW lanes lock to the
            # queue of their first user, so the 8-long pattern must repeat).
            QPAT = qpat or [[0, 1, 2, 3], [2, 3, 0, 1]]

            def body():
                col = 0
                for b in range(NBLK):
                    g = gpool.tile([P, g_max, D], mybir.dt.float16, tag="g")
                    off = 0
                    for q in range(NQ):
                        gq = g_sched[b][q]
                        win = table.ap()[BASES[q] : BASES[q] + WIN, :D]
                        _manual_dma_gather(
                            nc,
                            g[:, off : off + gq, :],
                            win,
                            idx_sb[:, col : col + wcols[b][q]],
                            gq * P,
                            D,
                            queue_num=QPAT[b % 2][q],
                            single_packet=(gq <= 8),
                        )
                        off += gq
                        col += wcols[b][q]
                    if mode == "gather":
                        continue
                    red = rpool.tile([P, D], mybir.dt.float32, tag="red")
                    if mode == "nored":
                        nc.vector.tensor_copy(red[:], g[:, 0, :])
                    else:
                        nc.vector.tensor_reduce(
                            out=red[:],
                            in_=g[:, : gtot[b], :].rearrange("p l d -> p d l"),
                            axis=mybir.AxisListType.X,
                            op=mybir.AluOpType.add,
                        )
                    o = rpool.tile([P, D], mybir.dt.float32, tag="o")
                    nc.scalar.activation(
                        o[:],
                        red[:],
                        mybir.ActivationFunctionType.Copy,
                        scale=invl_sb[:, b : b + 1],
                    )
                    nc.sync.dma_start(out.ap()[b], o[:])

            if reps == 1:
                body()
            else:
                with tc.For_i(0, reps, 1):
                    body()

    nc.compile()
    return nc


def _dev_table(table):
    """fp16 device table [DEVROWS, 128]: zero sentinel at each window base,
    original row r at device position devpos[r]."""
    t16 = np.asarray(table, dtype=np.float32).astype(np.float16)
    dev = np.zeros((DEVROWS, P), np.float16)
    devpos = np.empty(V, np.int64)
    src = 0
    for pos in range(DEVROWS):
        if pos in BASES:
            continue  # zero sentinel
        dev[pos, :D] = t16[src]
        devpos[src] = pos
        src += 1
    assert src == V
    return dev, devpos


def _balance_passes(devrows_sample):
    """Assign each device-row index to a feasible pass, minimizing the max
    per-pass count. Windows overlap only adjacently, so flexible indices sit
    on edges of a path -> exact min-max via binary search + left-greedy
    water-filling. Returns list of NQ lists of window-relative indices."""
    fixed = [[] for _ in range(NQ)]
    flex = [[] for _ in range(NQ - 1)]  # flex[e]: feasible in passes e, e+1
    for d in devrows_sample:
        feas = [q for q in range(NQ) if BASES[q] <= d < BASES[q] + WIN]
        if len(feas) == 1:
            fixed[feas[0]].append(d)
        else:
            flex[min(feas)].append(d)
    f = [len(x) for x in fixed]
    x = [len(e) for e in flex]

    def feasible(T):
        carry = 0  # edge flex forced rightward into the current pass
        for q in range(NQ):
            load = f[q] + carry
            if load > T:
                return False
            carry = max(0, x[q] - (T - load)) if q < NQ - 1 else 0
        return True

    lo = max(1, -(-len(devrows_sample) // NQ))
    hi = max(lo, max(f[q] + (x[q - 1] if q else 0) + (x[q] if q < NQ - 1 else 0)
                     for q in range(NQ)))
    while lo < hi:
        mid = (lo + hi) // 2
        if feasible(mid):
            hi = mid
        else:
            lo = mid + 1
    T = lo

    groups = [[d - BASES[q] for d in fixed[q]] for q in range(NQ)]
    carry_items: list = []
    for q in range(NQ):
        groups[q].extend(d - BASES[q] for d in carry_items)
        carry_items = []
        if q < NQ - 1:
            take = min(len(flex[q]), max(0, T - len(groups[q])))
            groups[q].extend(d - BASES[q] for d in flex[q][:take])
            carry_items = flex[q][take:]
    assert not carry_items
    return groups


def preprocess(table, indices, lengths):
    """Host prep. Returns (in_maps, g_sched, order) where order[r] is the
    original sample id at global dealt rank r."""
    dev, devpos = _dev_table(table)

    idx_np = np.asarray(indices, dtype=np.int64)  # [B, L]
    lens = np.asarray(lengths).astype(np.int64)  # [B]
    inv_len = (1.0 / np.maximum(lens, 1)).astype(np.float32)

    # per-sample pass groups (window-relative indices)
    sample_groups = []
    cnt = np.zeros((B, NQ), np.int64)
    for s in range(B):
        drows = devpos[idx_np[s, : lens[s]]]
        sample_groups.append(_balance_passes(drows))
        cnt[s] = [len(g) for g in sample_groups[s]]

    # greedy deal: assign samples to the 16 global blocks (1024 each) to
    # minimize the per-block per-pass maxima; rank r -> (block r//1024,
    # core (r%1024)//128, partition r%128)
    key = cnt.max(1) * 64 + lens
    pool = np.argsort(-key, kind="stable")
    gmax = np.zeros((NBLK, NQ), np.int64)
    fill = np.zeros(NBLK, np.int64)
    assign = np.empty(B, np.int64)
    for s in pool:
        best, bc = -1, None
        for b in range(NBLK):
            if fill[b] >= 1024:
                continue
            cost = np.maximum(gmax[b], cnt[s]).sum() - gmax[b].sum()
            if bc is None or cost < bc:
                best, bc = b, cost
        assign[s] = best
        gmax[best] = np.maximum(gmax[best], cnt[s])
        fill[best] += 1
    order = np.concatenate([pool[assign[pool] == b] for b in range(NBLK)])

    g_sched = [[int(x) for x in np.maximum(gmax[b], 1)] for b in range(NBLK)]

    wcols = [[g * P // 16 for g in r] for r in g_sched]
    WC = sum(sum(r) for r in wcols)

    in_maps = []
    for c in range(NCORES):
        idx16 = np.zeros((P, WC), np.int16)
        invl_dev = np.empty((P, NBLK), np.float32)
        col = 0
        for b in range(NBLK):
            ranks = order[b * 1024 + c * P : b * 1024 + (c + 1) * P]
            invl_dev[:, b] = inv_len[ranks]
            for q in range(NQ):
                gq = g_sched[b][q]
                blk = np.zeros((P, gq), np.int16)  # sentinel rel idx 0
                for p, s in enumerate(ranks):
                    grp = sample_groups[s][q]
                    blk[p, : len(grp)] = grp
                # stream order i = c*128 + p -> wrap int16 [16, nidx/16] x8
                flat = blk.T.ravel()  # [gq*128]
                w = flat.reshape(gq * P // 16, 16).T  # [16, cols]
                nw = wcols[b][q]
                idx16[:, col : col + nw] = np.tile(w, (8, 1))
                col += nw
        in_maps.append(
            {
                "table": dev,
                "idx": np.ascontiguousarray(idx16),
                "inv_len": np.ascontiguousarray(invl_dev),
            }
        )
    return in_maps, g_sched, order


def kernel(table, indices, lengths):
    in_maps, g_sched, order = preprocess(table, indices, lengths)
    key = tuple(tuple(r) for r in g_sched)
    nc = _CACHE.get(key)
    if nc is None:
        nc = _CACHE[key] = build(g_sched)
    res = bass_utils.run_bass_kernel_spmd(nc, in_maps, core_ids=list(range(NCORES)))
    full = np.empty((B, D), np.float32)
    for b in range(NBLK):
        for c in range(NCORES):
            ranks = order[b * 1024 + c * P : b * 1024 + (c + 1) * P]
            full[ranks] = res.results[c]["out"][b]
    return full


# revision 29
# speedup vs baseline: 1.5908x; 1.5908x over previous
"""EmbeddingBag-mean (padded ragged gather + masked mean) on 8 Trainium2 cores.

Strategy (data-parallel over batch):
  - Each of the 8 cores owns B/8 = 2048 samples; the embedding table is
    replicated to every core's HBM as fp16, rows padded to 128 elements
    (256 B stride) so the MoE `dma_gather` ucode (stride in 256 B units,
    int16 indices) can gather single 128 B rows.
  - int16 indices only reach 32768 rows, so the gather runs as 4 passes
    over overlapping 32768-row windows (bases ~22412 apart).  A zero
    sentinel row sits at each window base (relative index 0).  Because the
    windows overlap, each sample distributes its indices among feasible
    passes to equalize its per-pass counts, which keeps the per-block
    per-pass slot maxima near len/4 (instead of len/4 + 3 sigma).
  - Samples are globally length-sorted and dealt to (block, core,
    partition) so each block of 128 partitions holds samples of nearly
    equal length; block b needs G[b][q] gather slots for pass q (max over
    the 8 cores' blocks; one SPMD module).  Pad slots -> sentinel row.
  - Device kernel (per core), per block of 128 samples:
      1. 4x dma_gather (one per pass, 4 SWDGE queues round-robin):
         g[p, off_q + j, :] = window_q[idx16[...], :64]   (128 B descs)
      2. one DVE tensor_reduce over all slot columns (fp16 in, fp32 out)
      3. ACT Copy-with-scale by 1/max(len,1) (per-partition scalar)
      4. DMA the [128, 64] fp32 block out
  - Host un-permutes the global deal and returns [B, 64] fp32.
"""

import numpy as np

try:
    import concourse.bacc as bacc
except ImportError:  # harness containers keep the repo at /opt/trn_rl_repo
    import sys

    sys.path.insert(0, "/opt/trn_rl_repo")
    import concourse.bacc as bacc

import concourse.bass as bass
import concourse.mybir as mybir
import concourse.tile as tile
from concourse import bass_utils

B, L, V, D = 16384, 50, 100000, 64
NCORES = 8
P = 128
BC = B // NCORES  # 2048 samples per core
NBLK = BC // P  # 16 blocks of 128 samples
NQ = 4  # gather passes (overlapping windows)
WIN = 32768  # int16-reachable rows per pass
DEVC = V + NQ  # core device rows: table + one zero sentinel per window

# wrap layout: windows tile a circle (the first WIN-S core rows are
# duplicated after the end), so the pass-exclusive mass is uniform instead
# of piling onto the outer windows.  Sentinel zero row at each window base.
_SPACING = -(-DEVC // NQ)  # ceil
BASES = [q * _SPACING for q in range(NQ)]
DEVROWS = BASES[-1] + WIN  # core + duplicated prefix
assert DEVROWS >= DEVC and _SPACING < WIN

_CACHE: dict = {}


def _manual_dma_gather(nc, out_ap, in_ap, idxs_ap, num_idxs, elem_size,
                       queue_num, single_packet):
    """dma_gather without the elem_size_bytes%256 assert (stride is 256B)."""
    gp = nc.gpsimd
    _in_ap = gp.lower_ap_dma(in_ap, for_custom_bir_dma=True)
    _idxs_ap = gp.lower_ap(idxs_ap)
    _out_ap = gp.lower_ap(out_ap)
    stride_bytes = in_ap.ap[0][0] * mybir.dt.size(in_ap.dtype)
    assert stride_bytes % 256 == 0
    return gp.add_instruction(
        mybir.InstDMAGatherAnt(
            name=nc.get_next_instruction_name(),
            ins=[*_in_ap, _idxs_ap, gp.lower_val_access(gp.to_reg(num_idxs))],
            outs=[_out_ap],
            transpose=False,
            num_idxs=num_idxs,
            elem_size=elem_size,
            stride_bytes_256=stride_bytes // 256,
            gen_mode=0,
            single_packet=single_packet,
            queue_num=queue_num,
            sbuf_tokens_per_rank=0,
            sbuf_free_dim_per_rank=0,
            sbuf_free_dim_pad_per_rank=0,
            sbuf_byte_offset=0,
        )
    )


def build(g_sched, reps: int = 1, mode: str = "full", qpat=None):
    """Build + compile the per-core Bass module.

    g_sched: [NBLK][NQ] gather slot counts (>=1 each).
    reps > 1 wraps the block loop in tc.For_i for slope timing.
    mode: "full" | "gather" (skip reduce/scale/out) | "nored" (skip reduce).
    """
    g_sched = [list(r) for r in g_sched]
    assert len(g_sched) == NBLK and all(len(r) == NQ for r in g_sched)
    gtot = [sum(r) for r in g_sched]
    g_max = max(gtot)
    # idx16 column layout: per (block, pass) a run of G*P/16 int16 columns
    wcols = [[g * P // 16 for g in r] for r in g_sched]
    WC = sum(sum(r) for r in wcols)

    nc = bacc.Bacc("TRN2", target_bir_lowering=False, debug=False,
                   num_swdge_queues=NQ)
    table = nc.dram_tensor("table", [DEVROWS, P], mybir.dt.float16,
                           kind="ExternalInput")
    idx = nc.dram_tensor("idx", [P, WC], mybir.dt.int16, kind="ExternalInput")
    inv_len = nc.dram_tensor("inv_len", [P, NBLK], mybir.dt.float32,
                             kind="ExternalInput")
    out = nc.dram_tensor("out", [NBLK, P, D], mybir.dt.float32,
                         kind="ExternalOutput")

    with tile.TileContext(nc) as tc:
        with (
            tc.tile_pool(name="const", bufs=1) as cpool,
            tc.tile_pool(name="gather", bufs=4) as gpool,
            tc.tile_pool(name="res", bufs=4) as rpool,
        ):
            idx_sb = cpool.tile([P, WC], mybir.dt.int16)
            nc.sync.dma_start(idx_sb[:], idx.ap())
            invl_sb = cpool.tile([P, NBLK], mybir.dt.float32)
            nc.sync.dma_start(invl_sb[:], inv_len.ap())

            # one gather per (block, pass).  Queue pattern alternates between
            # blocks so the big outer passes split evenly across queues, while
            # staying periodic in issue order (Tile's DMASW lanes lock to the
            # queue of their first user, so the 8-long pattern must repeat).
            QPAT = qpat or [[0, 1, 2, 3], [2, 3, 0, 1]]

            def body():
                col = 0
                for b in range(NBLK):
                    g = gpool.tile([P, g_max, D], mybir.dt.float16, tag="g")
                    off = 0
                    for q in range(NQ):
                        gq = g_sched[b][q]
                        win = table.ap()[BASES[q] : BASES[q] + WIN, :D]
                        _manual_dma_gather(
                            nc,
                            g[:, off : off + gq, :],
                            win,
                            idx_sb[:, col : col + wcols[b][q]],
                            gq * P,
                            D,
                            queue_num=QPAT[b % 2][q],
                            single_packet=(gq <= 8),
                        )
                        off += gq
                        col += wcols[b][q]
                    if mode == "gather":
                        continue
                    red = rpool.tile([P, D], mybir.dt.float32, tag="red")
                    if mode == "nored":
                        nc.vector.tensor_copy(red[:], g[:, 0, :])
                    else:
                        nc.vector.tensor_reduce(
                            out=red[:],
                            in_=g[:, : gtot[b], :].rearrange("p l d -> p d l"),
                            axis=mybir.AxisListType.X,
                            op=mybir.AluOpType.add,
                        )
                    o = rpool.tile([P, D], mybir.dt.float32, tag="o")
                    nc.scalar.activation(
                        o[:],
                        red[:],
                        mybir.ActivationFunctionType.Copy,
                        scale=invl_sb[:, b : b + 1],
                    )
                    nc.sync.dma_start(out.ap()[b], o[:])

            if reps == 1:
                body()
            else:
                with tc.For_i(0, reps, 1):
                    body()

    nc.compile()
    return nc


def _dev_table(table):
    """fp16 device table [DEVROWS, 128]: zero sentinel at each window base,
    original row r at core position devpos[r], first DEVROWS-DEVC core rows
    duplicated after the end (wrap)."""
    t16 = np.asarray(table, dtype=np.float32).astype(np.float16)
    dev = np.zeros((DEVROWS, P), np.float16)
    devpos = np.empty(V, np.int64)
    src = 0
    bset = set(BASES)
    for pos in range(DEVC):
        if pos in bset:
            continue  # zero sentinel
        dev[pos, :D] = t16[src]
        devpos[src] = pos
        src += 1
    assert src == V
    dev[DEVC:] = dev[: DEVROWS - DEVC]
    return dev, devpos


def _feasible_rels(d):
    """[(pass, window-relative idx)] for core position d, incl. wrap copy."""
    out = [(q, d - BASES[q]) for q in range(NQ)
           if BASES[q] <= d < BASES[q] + WIN]
    if d + DEVC < DEVROWS:  # duplicated prefix: reachable from the last pass
        out.append((NQ - 1, d + DEVC - BASES[NQ - 1]))
    return out


def _balance_passes(devrows_sample):
    """Assign each device-row index to a feasible pass, minimizing the max
    per-pass count. Windows overlap adjacently on a circle, so flexible
    indices sit on edges of a cycle -> binary search on T; for each T scan
    the wrap-edge split and run left-greedy water-filling on the path.
    Returns list of NQ lists of window-relative indices."""
    fixed = [[] for _ in range(NQ)]
    flex = [[] for _ in range(NQ)]  # edge e: passes (e, (e+1)%NQ)
    for d in devrows_sample:
        feas = _feasible_rels(d)
        if len(feas) == 1:
            fixed[feas[0][0]].append(feas[0][1])
        else:
            qs = sorted(q for q, _ in feas)
            e = NQ - 1 if qs == [0, NQ - 1] else qs[0]
            flex[e].append(dict(feas))
    f = [len(g) for g in fixed]
    x = [len(e) for e in flex]

    def path_ok(T, t3):
        # t3 wrap items to pass NQ-1; the rest (x[NQ-1]-t3) preload pass 0
        takes = [0] * (NQ - 1)
        carry = x[NQ - 1] - t3
        for q in range(NQ - 1):
            load = f[q] + carry
            if load > T:
                return None
            takes[q] = min(x[q], T - load)
            carry = x[q] - takes[q]
        if f[NQ - 1] + carry + t3 > T:
            return None
        return takes

    lo = max(1, -(-len(devrows_sample) // NQ))
    hi = max(lo, max(f) + sum(x))
    best = None
    while lo < hi:
        mid = (lo + hi) // 2
        sol = next((
            (t3, tk) for t3 in range(x[NQ - 1] + 1)
            if (tk := path_ok(mid, t3)) is not None), None)
        if sol is not None:
            hi = mid
        else:
            lo = mid + 1
    T = lo
    t3, takes = next((t3, tk) for t3 in range(x[NQ - 1] + 1)
                     if (tk := path_ok(T, t3)) is not None)

    groups = [list(fixed[q]) for q in range(NQ)]
    # wrap edge: t3 items to pass NQ-1, rest to pass 0
    for i, item in enumerate(flex[NQ - 1]):
        q = NQ - 1 if i < t3 else 0
        groups[q].append(item[q])
    for e in range(NQ - 1):
        for i, item in enumerate(flex[e]):
            q = e if i < takes[e] else e + 1
            groups[q].append(item[q])
    return groups


def preprocess(table, indices, lengths):
    """Host prep. Returns (in_maps, g_sched, order) where order[r] is the
    original sample id at global dealt rank r."""
    dev, devpos = _dev_table(table)

    idx_np = np.asarray(indices, dtype=np.int64)  # [B, L]
    lens = np.asarray(lengths).astype(np.int64)  # [B]
    inv_len = (1.0 / np.maximum(lens, 1)).astype(np.float32)

    # per-sample pass groups (window-relative indices)
    sample_groups = []
    cnt = np.zeros((B, NQ), np.int64)
    for s in range(B):
        drows = devpos[idx_np[s, : lens[s]]]
        sample_groups.append(_balance_passes(drows))
        cnt[s] = [len(g) for g in sample_groups[s]]

    # greedy deal: assign samples to the 16 global blocks (1024 each) to
    # minimize the per-block per-pass maxima; rank r -> (block r//1024,
    # core (r%1024)//128, partition r%128)
    key = cnt.max(1) * 64 + lens
    pool = np.argsort(-key, kind="stable")
    gmax = np.zeros((NBLK, NQ), np.int64)
    fill = np.zeros(NBLK, np.int64)
    assign = np.empty(B, np.int64)
    for s in pool:
        best, bc = -1, None
        for b in range(NBLK):
            if fill[b] >= 1024:
                continue
            cost = np.maximum(gmax[b], cnt[s]).sum() - gmax[b].sum()
            if bc is None or cost < bc:
                best, bc = b, cost
        assign[s] = best
        gmax[best] = np.maximum(gmax[best], cnt[s])
        fill[best] += 1
    order = np.concatenate([pool[assign[pool] == b] for b in range(NBLK)])

    g_sched = [[int(x) for x in np.maximum(gmax[b], 1)] for b in range(NBLK)]

    wcols = [[g * P // 16 for g in r] for r in g_sched]
    WC = sum(sum(r) for r in wcols)

    in_maps = []
    for c in range(NCORES):
        idx16 = np.zeros((P, WC), np.int16)
        invl_dev = np.empty((P, NBLK), np.float32)
        col = 0
        for b in range(NBLK):
            ranks = order[b * 1024 + c * P : b * 1024 + (c + 1) * P]
            invl_dev[:, b] = inv_len[ranks]
            for q in range(NQ):
                gq = g_sched[b][q]
                blk = np.zeros((P, gq), np.int16)  # sentinel rel idx 0
                for p, s in enumerate(ranks):
                    grp = sample_groups[s][q]
                    blk[p, : len(grp)] = grp
                # stream order i = c*128 + p -> wrap int16 [16, nidx/16] x8
                flat = blk.T.ravel()  # [gq*128]
                w = flat.reshape(gq * P // 16, 16).T  # [16, cols]
                nw = wcols[b][q]
                idx16[:, col : col + nw] = np.tile(w, (8, 1))
                col += nw
        in_maps.append(
            {
                "table": dev,
                "idx": np.ascontiguousarray(idx16),
                "inv_len": np.ascontiguousarray(invl_dev),
            }
        )
    return in_maps, g_sched, order


def kernel(table, indices, lengths):
    in_maps, g_sched, order = preprocess(table, indices, lengths)
    key = tuple(tuple(r) for r in g_sched)
    nc = _CACHE.get(key)
    if nc is None:
        nc = _CACHE[key] = build(g_sched)
    res = bass_utils.run_bass_kernel_spmd(nc, in_maps, core_ids=list(range(NCORES)))
    full = np.empty((B, D), np.float32)
    for b in range(NBLK):
        for c in range(NCORES):
            ranks = order[b * 1024 + c * P : b * 1024 + (c + 1) * P]
            full[ranks] = res.results[c]["out"][b]
    return full


# revision 32
# speedup vs baseline: 1.6611x; 1.0442x over previous
"""EmbeddingBag-mean (padded ragged gather + masked mean) on 8 Trainium2 cores.

Strategy (data-parallel over batch):
  - Each of the 8 cores owns B/8 = 2048 samples; the embedding table is
    replicated to every core's HBM as fp16, rows padded to 128 elements
    (256 B stride) so the MoE `dma_gather` ucode (stride in 256 B units,
    int16 indices) can gather single 128 B rows.
  - int16 indices only reach 32768 rows, so the gather runs as 4 passes
    over overlapping 32768-row windows (bases ~22412 apart).  A zero
    sentinel row sits at each window base (relative index 0).  Because the
    windows overlap, each sample distributes its indices among feasible
    passes to equalize its per-pass counts, which keeps the per-block
    per-pass slot maxima near len/4 (instead of len/4 + 3 sigma).
  - Samples are globally length-sorted and dealt to (block, core,
    partition) so each block of 128 partitions holds samples of nearly
    equal length; block b needs G[b][q] gather slots for pass q (max over
    the 8 cores' blocks; one SPMD module).  Pad slots -> sentinel row.
  - Device kernel (per core), per block of 128 samples:
      1. 4x dma_gather (one per pass, 4 SWDGE queues round-robin):
         g[p, off_q + j, :] = window_q[idx16[...], :64]   (128 B descs)
      2. one DVE tensor_reduce over all slot columns (fp16 in, fp32 out)
      3. ACT Copy-with-scale by 1/max(len,1) (per-partition scalar)
      4. DMA the [128, 64] fp32 block out
  - Host un-permutes the global deal and returns [B, 64] fp32.
"""

import numpy as np

try:
    import concourse.bacc as bacc
except ImportError:  # harness containers keep the repo at /opt/trn_rl_repo
    import sys

    sys.path.insert(0, "/opt/trn_rl_repo")
    import concourse.bacc as bacc

import concourse.bass as bass
import concourse.mybir as mybir
import concourse.tile as tile
from concourse import bass_utils

B, L, V, D = 16384, 50, 100000, 64
NCORES = 8
P = 128
BC = B // NCORES  # 2048 samples per core
NBLK = BC // P  # 16 blocks of 128 samples
NQ = 5  # gather passes (overlapping windows)
WIN = 32768  # int16-reachable rows per pass
DEVC = V + NQ  # core device rows: table + one zero sentinel per window

# wrap layout: windows tile a circle (the first WIN-S core rows are
# duplicated after the end), so the pass-exclusive mass is uniform instead
# of piling onto the outer windows.  Sentinel zero row at each window base.
_SPACING = -(-DEVC // NQ)  # ceil
BASES = [q * _SPACING for q in range(NQ)]
DEVROWS = BASES[-1] + WIN  # core + duplicated prefix
assert DEVROWS >= DEVC and _SPACING < WIN

_CACHE: dict = {}


def _manual_dma_gather(nc, out_ap, in_ap, idxs_ap, num_idxs, elem_size,
                       queue_num, single_packet):
    """dma_gather without the elem_size_bytes%256 assert (stride is 256B)."""
    gp = nc.gpsimd
    _in_ap = gp.lower_ap_dma(in_ap, for_custom_bir_dma=True)
    _idxs_ap = gp.lower_ap(idxs_ap)
    _out_ap = gp.lower_ap(out_ap)
    stride_bytes = in_ap.ap[0][0] * mybir.dt.size(in_ap.dtype)
    assert stride_bytes % 256 == 0
    return gp.add_instruction(
        mybir.InstDMAGatherAnt(
            name=nc.get_next_instruction_name(),
            ins=[*_in_ap, _idxs_ap, gp.lower_val_access(gp.to_reg(num_idxs))],
            outs=[_out_ap],
            transpose=False,
            num_idxs=num_idxs,
            elem_size=elem_size,
            stride_bytes_256=stride_bytes // 256,
            gen_mode=0,
            single_packet=single_packet,
            queue_num=queue_num,
            sbuf_tokens_per_rank=0,
            sbuf_free_dim_per_rank=0,
            sbuf_free_dim_pad_per_rank=0,
            sbuf_byte_offset=0,
        )
    )


def build(g_sched, reps: int = 1, mode: str = "full", qpat=None):
    """Build + compile the per-core Bass module.

    g_sched: [NBLK][NQ] gather slot counts (>=1 each).
    reps > 1 wraps the block loop in tc.For_i for slope timing.
    mode: "full" | "gather" (skip reduce/scale/out) | "nored" (skip reduce).
    """
    g_sched = [list(r) for r in g_sched]
    assert len(g_sched) == NBLK and all(len(r) == NQ for r in g_sched)
    gtot = [sum(r) for r in g_sched]
    g_max = max(gtot)
    # idx16 column layout: per (block, pass) a run of G*P/16 int16 columns
    wcols = [[g * P // 16 for g in r] for r in g_sched]
    WC = sum(sum(r) for r in wcols)

    nc = bacc.Bacc("TRN2", target_bir_lowering=False, debug=False,
                   num_swdge_queues=4)
    table = nc.dram_tensor("table", [DEVROWS, P], mybir.dt.float16,
                           kind="ExternalInput")
    idx = nc.dram_tensor("idx", [P, WC], mybir.dt.int16, kind="ExternalInput")
    inv_len = nc.dram_tensor("inv_len", [P, NBLK], mybir.dt.float32,
                             kind="ExternalInput")
    out = nc.dram_tensor("out", [NBLK, P, D], mybir.dt.float32,
                         kind="ExternalOutput")

    with tile.TileContext(nc) as tc:
        with (
            tc.tile_pool(name="const", bufs=1) as cpool,
            tc.tile_pool(name="gather", bufs=4) as gpool,
            tc.tile_pool(name="res", bufs=4) as rpool,
        ):
            idx_sb = cpool.tile([P, WC], mybir.dt.int16)
            nc.sync.dma_start(idx_sb[:], idx.ap())
            invl_sb = cpool.tile([P, NBLK], mybir.dt.float32)
            nc.sync.dma_start(invl_sb[:], inv_len.ap())

            # one gather per (block, pass), queue = issue counter % 4:
            # strictly periodic in issue order (Tile's DMASW lanes lock to
            # the queue of their first user), and with NQ=5 passes the
            # phase rotates every block, balancing pass sizes across queues.
            ictr = [0]

            def body():
                col = 0
                for b in range(NBLK):
                    g = gpool.tile([P, g_max, D], mybir.dt.float16, tag="g")
                    off = 0
                    for q in range(NQ):
                        gq = g_sched[b][q]
                        win = table.ap()[BASES[q] : BASES[q] + WIN, :D]
                        _manual_dma_gather(
                            nc,
                            g[:, off : off + gq, :],
                            win,
                            idx_sb[:, col : col + wcols[b][q]],
                            gq * P,
                            D,
                            queue_num=ictr[0] % 4,
                            single_packet=(gq <= 8),
                        )
                        ictr[0] += 1
                        off += gq
                        col += wcols[b][q]
                    if mode == "gather":
                        continue
                    red = rpool.tile([P, D], mybir.dt.float32, tag="red")
                    if mode == "nored":
                        nc.vector.tensor_copy(red[:], g[:, 0, :])
                    else:
                        nc.vector.tensor_reduce(
                            out=red[:],
                            in_=g[:, : gtot[b], :].rearrange("p l d -> p d l"),
                            axis=mybir.AxisListType.X,
                            op=mybir.AluOpType.add,
                        )
                    o = rpool.tile([P, D], mybir.dt.float32, tag="o")
                    nc.scalar.activation(
                        o[:],
                        red[:],
                        mybir.ActivationFunctionType.Copy,
                        scale=invl_sb[:, b : b + 1],
                    )
                    nc.sync.dma_start(out.ap()[b], o[:])

            if reps == 1:
                body()
            else:
                with tc.For_i(0, reps, 1):
                    body()

    nc.compile()
    return nc


def _dev_table(table):
    """fp16 device table [DEVROWS, 128]: zero sentinel at each window base,
    original row r at core position devpos[r], first DEVROWS-DEVC core rows
    duplicated after the end (wrap)."""
    t16 = np.asarray(table, dtype=np.float32).astype(np.float16)
    dev = np.zeros((DEVROWS, P), np.float16)
    devpos = np.empty(V, np.int64)
    src = 0
    bset = set(BASES)
    for pos in range(DEVC):
        if pos in bset:
            continue  # zero sentinel
        dev[pos, :D] = t16[src]
        devpos[src] = pos
        src += 1
    assert src == V
    dev[DEVC:] = dev[: DEVROWS - DEVC]
    return dev, devpos


def _feasible_rels(d):
    """[(pass, window-relative idx)] for core position d, incl. wrap copy."""
    out = [(q, d - BASES[q]) for q in range(NQ)
           if BASES[q] <= d < BASES[q] + WIN]
    if d + DEVC < DEVROWS:  # duplicated prefix: reachable from the last pass
        out.append((NQ - 1, d + DEVC - BASES[NQ - 1]))
    return out


def _balance_passes(devrows_sample):
    """Assign each device-row index to a feasible pass, minimizing the max
    per-pass count. Windows overlap adjacently on a circle, so flexible
    indices sit on edges of a cycle -> binary search on T; for each T scan
    the wrap-edge split and run left-greedy water-filling on the path.
    Returns list of NQ lists of window-relative indices."""
    fixed = [[] for _ in range(NQ)]
    flex = [[] for _ in range(NQ)]  # edge e: passes (e, (e+1)%NQ)
    for d in devrows_sample:
        feas = _feasible_rels(d)
        if len(feas) == 1:
            fixed[feas[0][0]].append(feas[0][1])
        else:
            qs = sorted(q for q, _ in feas)
            e = NQ - 1 if qs == [0, NQ - 1] else qs[0]
            flex[e].append(dict(feas))
    f = [len(g) for g in fixed]
    x = [len(e) for e in flex]

    def path_ok(T, t3):
        # t3 wrap items to pass NQ-1; the rest (x[NQ-1]-t3) preload pass 0
        takes = [0] * (NQ - 1)
        carry = x[NQ - 1] - t3
        for q in range(NQ - 1):
            load = f[q] + carry
            if load > T:
                return None
            takes[q] = min(x[q], T - load)
            carry = x[q] - takes[q]
        if f[NQ - 1] + carry + t3 > T:
            return None
        return takes

    lo = max(1, -(-len(devrows_sample) // NQ))
    hi = max(lo, max(f) + sum(x))
    best = None
    while lo < hi:
        mid = (lo + hi) // 2
        sol = next((
            (t3, tk) for t3 in range(x[NQ - 1] + 1)
            if (tk := path_ok(mid, t3)) is not None), None)
        if sol is not None:
            hi = mid
        else:
            lo = mid + 1
    T = lo
    t3, takes = next((t3, tk) for t3 in range(x[NQ - 1] + 1)
                     if (tk := path_ok(T, t3)) is not None)

    groups = [list(fixed[q]) for q in range(NQ)]
    # wrap edge: t3 items to pass NQ-1, rest to pass 0
    for i, item in enumerate(flex[NQ - 1]):
        q = NQ - 1 if i < t3 else 0
        groups[q].append(item[q])
    for e in range(NQ - 1):
        for i, item in enumerate(flex[e]):
            q = e if i < takes[e] else e + 1
            groups[q].append(item[q])
    return groups


def preprocess(table, indices, lengths):
    """Host prep. Returns (in_maps, g_sched, order) where order[r] is the
    original sample id at global dealt rank r."""
    dev, devpos = _dev_table(table)

    idx_np = np.asarray(indices, dtype=np.int64)  # [B, L]
    lens = np.asarray(lengths).astype(np.int64)  # [B]
    inv_len = (1.0 / np.maximum(lens, 1)).astype(np.float32)

    # per-sample pass groups (window-relative indices)
    sample_groups = []
    cnt = np.zeros((B, NQ), np.int64)
    for s in range(B):
        drows = devpos[idx_np[s, : lens[s]]]
        sample_groups.append(_balance_passes(drows))
        cnt[s] = [len(g) for g in sample_groups[s]]

    # greedy deal: assign samples to the 16 global blocks (1024 each) to
    # minimize the per-block per-pass maxima; rank r -> (block r//1024,
    # core (r%1024)//128, partition r%128)
    key = cnt.max(1) * 64 + lens
    pool = np.argsort(-key, kind="stable")
    gmax = np.zeros((NBLK, NQ), np.int64)
    fill = np.zeros(NBLK, np.int64)
    assign = np.empty(B, np.int64)
    for s in pool:
        best, bc = -1, None
        for b in range(NBLK):
            if fill[b] >= 1024:
                continue
            cost = np.maximum(gmax[b], cnt[s]).sum() - gmax[b].sum()
            if bc is None or cost < bc:
                best, bc = b, cost
        assign[s] = best
        gmax[best] = np.maximum(gmax[best], cnt[s])
        fill[best] += 1
    order = np.concatenate([pool[assign[pool] == b] for b in range(NBLK)])

    g_sched = [[int(x) for x in np.maximum(gmax[b], 1)] for b in range(NBLK)]

    wcols = [[g * P // 16 for g in r] for r in g_sched]
    WC = sum(sum(r) for r in wcols)

    in_maps = []
    for c in range(NCORES):
        idx16 = np.zeros((P, WC), np.int16)
        invl_dev = np.empty((P, NBLK), np.float32)
        col = 0
        for b in range(NBLK):
            ranks = order[b * 1024 + c * P : b * 1024 + (c + 1) * P]
            invl_dev[:, b] = inv_len[ranks]
            for q in range(NQ):
                gq = g_sched[b][q]
                blk = np.zeros((P, gq), np.int16)  # sentinel rel idx 0
                for p, s in enumerate(ranks):
                    grp = sample_groups[s][q]
                    blk[p, : len(grp)] = grp
                # stream order i = c*128 + p -> wrap int16 [16, nidx/16] x8
                flat = blk.T.ravel()  # [gq*128]
                w = flat.reshape(gq * P // 16, 16).T  # [16, cols]
                nw = wcols[b][q]
                idx16[:, col : col + nw] = np.tile(w, (8, 1))
                col += nw
        in_maps.append(
            {
                "table": dev,
                "idx": np.ascontiguousarray(idx16),
                "inv_len": np.ascontiguousarray(invl_dev),
            }
        )
    return in_maps, g_sched, order


def kernel(table, indices, lengths):
    in_maps, g_sched, order = preprocess(table, indices, lengths)
    key = tuple(tuple(r) for r in g_sched)
    nc = _CACHE.get(key)
    if nc is None:
        nc = _CACHE[key] = build(g_sched)
    res = bass_utils.run_bass_kernel_spmd(nc, in_maps, core_ids=list(range(NCORES)))
    full = np.empty((B, D), np.float32)
    for b in range(NBLK):
        for c in range(NCORES):
            ranks = order[b * 1024 + c * P : b * 1024 + (c + 1) * P]
            full[ranks] = res.results[c]["out"][b]
    return full


# revision 35
# speedup vs baseline: 1.6872x; 1.0157x over previous
"""EmbeddingBag-mean (padded ragged gather + masked mean) on 8 Trainium2 cores.

Strategy (data-parallel over batch):
  - Each of the 8 cores owns B/8 = 2048 samples; the embedding table is
    replicated to every core's HBM as fp16, rows padded to 128 elements
    (256 B stride) so the MoE `dma_gather` ucode (stride in 256 B units,
    int16 indices) can gather single 128 B rows.
  - int16 indices only reach 32768 rows, so the gather runs as NQ=5 passes
    over overlapping 32768-row windows whose bases (spacing 20001) tile a
    circle: the first WIN-spacing core rows are duplicated after the table
    end, so every pass has the same (small) exclusive index mass.  A zero
    sentinel row sits at each window base (relative index 0).  Flexible
    indices (in two windows' overlap) sit on edges of a cycle; per sample,
    exact min-max water-filling (binary search + wrap-edge scan) equalizes
    per-pass counts to ~ceil(len/5).
  - Samples are dealt greedily to 16 global blocks of 1024 (8 cores x 128
    partitions) minimizing the per-block per-pass slot maxima G[b][q]
    (shared across cores; one SPMD module).  Pad slots -> sentinel row.
  - Device kernel (per core), per block of 128 samples:
      1. NQ dma_gather (queue = issue counter % 4, periodic for Tile's
         DMASW lanes): g[p, off_q + j, :] = window_q[idx16[...], :64]
         (128 B descriptors; manual instruction to bypass the bass-level
         elem_size%256 assert, which the ucode only needs for transpose)
      2. one DVE tensor_reduce over all slot columns (fp16 in, fp32 out)
      3. ACT Copy-with-scale by 1/max(len,1) (per-partition scalar)
      4. DMA the [128, 64] fp32 block out
  - Host un-permutes the global deal and returns [B, 64] fp32.
"""

import numpy as np

try:
    import concourse.bacc as bacc
except ImportError:  # harness containers keep the repo at /opt/trn_rl_repo
    import sys

    sys.path.insert(0, "/opt/trn_rl_repo")
    import concourse.bacc as bacc

import concourse.bass as bass
import concourse.mybir as mybir
import concourse.tile as tile
from concourse import bass_utils

B, L, V, D = 16384, 50, 100000, 64
NCORES = 8
P = 128
BC = B // NCORES  # 2048 samples per core
NBLK = BC // P  # 16 blocks of 128 samples
NQ = 5  # gather passes (overlapping windows)
WIN = 32768  # int16-reachable rows per pass
DEVC = V + NQ  # core device rows: table + one zero sentinel per window

# wrap layout: windows tile a circle (the first WIN-S core rows are
# duplicated after the end), so the pass-exclusive mass is uniform instead
# of piling onto the outer windows.  Sentinel zero row at each window base.
_SPACING = -(-DEVC // NQ)  # ceil
BASES = [q * _SPACING for q in range(NQ)]
DEVROWS = BASES[-1] + WIN  # core + duplicated prefix
assert DEVROWS >= DEVC and _SPACING < WIN

_CACHE: dict = {}


def _manual_dma_gather(nc, out_ap, in_ap, idxs_ap, num_idxs, elem_size,
                       queue_num, single_packet):
    """dma_gather without the elem_size_bytes%256 assert (stride is 256B)."""
    gp = nc.gpsimd
    _in_ap = gp.lower_ap_dma(in_ap, for_custom_bir_dma=True)
    _idxs_ap = gp.lower_ap(idxs_ap)
    _out_ap = gp.lower_ap(out_ap)
    stride_bytes = in_ap.ap[0][0] * mybir.dt.size(in_ap.dtype)
    assert stride_bytes % 256 == 0
    return gp.add_instruction(
        mybir.InstDMAGatherAnt(
            name=nc.get_next_instruction_name(),
            ins=[*_in_ap, _idxs_ap, gp.lower_val_access(gp.to_reg(num_idxs))],
            outs=[_out_ap],
            transpose=False,
            num_idxs=num_idxs,
            elem_size=elem_size,
            stride_bytes_256=stride_bytes // 256,
            gen_mode=0,
            single_packet=single_packet,
            queue_num=queue_num,
            sbuf_tokens_per_rank=0,
            sbuf_free_dim_per_rank=0,
            sbuf_free_dim_pad_per_rank=0,
            sbuf_byte_offset=0,
        )
    )


def build(g_sched, reps: int = 1, mode: str = "full", qpat=None):
    """Build + compile the per-core Bass module.

    g_sched: [NBLK][NQ] gather slot counts (>=1 each).
    reps > 1 wraps the block loop in tc.For_i for slope timing.
    mode: "full" | "gather" (skip reduce/scale/out) | "nored" (skip reduce).
    """
    g_sched = [list(r) for r in g_sched]
    assert len(g_sched) == NBLK and all(len(r) == NQ for r in g_sched)
    gtot = [sum(r) for r in g_sched]
    g_max = max(gtot)
    # idx16 column layout: per (block, pass) a run of G*P/16 int16 columns
    wcols = [[g * P // 16 for g in r] for r in g_sched]
    WC = sum(sum(r) for r in wcols)

    nc = bacc.Bacc("TRN2", target_bir_lowering=False, debug=False,
                   num_swdge_queues=4)
    table = nc.dram_tensor("table", [DEVROWS, P], mybir.dt.float16,
                           kind="ExternalInput")
    idx = nc.dram_tensor("idx", [P, WC], mybir.dt.int16, kind="ExternalInput")
    inv_len = nc.dram_tensor("inv_len", [P, NBLK], mybir.dt.float32,
                             kind="ExternalInput")
    out = nc.dram_tensor("out", [NBLK, P, D], mybir.dt.float32,
                         kind="ExternalOutput")

    with tile.TileContext(nc) as tc:
        with (
            tc.tile_pool(name="const", bufs=1) as cpool,
            tc.tile_pool(name="gather", bufs=4) as gpool,
            tc.tile_pool(name="res", bufs=4) as rpool,
        ):
            idx_sb = cpool.tile([P, WC], mybir.dt.int16)
            nc.sync.dma_start(idx_sb[:], idx.ap())
            invl_sb = cpool.tile([P, NBLK], mybir.dt.float32)
            nc.sync.dma_start(invl_sb[:], inv_len.ap())

            # one gather per (block, pass), queue = issue counter % 4:
            # strictly periodic in issue order (Tile's DMASW lanes lock to
            # the queue of their first user), and with NQ=5 passes the
            # phase rotates every block, balancing pass sizes across queues.
            ictr = [0]

            def body():
                col = 0
                for b in range(NBLK):
                    g = gpool.tile([P, g_max, D], mybir.dt.float16, tag="g")
                    off = 0
                    for q in range(NQ):
                        gq = g_sched[b][q]
                        win = table.ap()[BASES[q] : BASES[q] + WIN, :D]
                        _manual_dma_gather(
                            nc,
                            g[:, off : off + gq, :],
                            win,
                            idx_sb[:, col : col + wcols[b][q]],
                            gq * P,
                            D,
                            queue_num=ictr[0] % 4,
                            single_packet=(gq <= 8),
                        )
                        ictr[0] += 1
                        off += gq
                        col += wcols[b][q]
                    if mode == "gather":
                        continue
                    red = rpool.tile([P, D], mybir.dt.float32, tag="red")
                    if mode == "nored":
                        nc.vector.tensor_copy(red[:], g[:, 0, :])
                    else:
                        nc.vector.tensor_reduce(
                            out=red[:],
                            in_=g[:, : gtot[b], :].rearrange("p l d -> p d l"),
                            axis=mybir.AxisListType.X,
                            op=mybir.AluOpType.add,
                        )
                    o = rpool.tile([P, D], mybir.dt.float32, tag="o")
                    nc.scalar.activation(
                        o[:],
                        red[:],
                        mybir.ActivationFunctionType.Copy,
                        scale=invl_sb[:, b : b + 1],
                    )
                    nc.sync.dma_start(out.ap()[b], o[:])

            if reps == 1:
                body()
            else:
                with tc.For_i(0, reps, 1):
                    body()

    nc.compile()
    return nc


def _dev_table(table):
    """fp16 device table [DEVROWS, 128]: zero sentinel at each window base,
    original row r at core position devpos[r], first DEVROWS-DEVC core rows
    duplicated after the end (wrap)."""
    t16 = np.asarray(table, dtype=np.float32).astype(np.float16)
    dev = np.zeros((DEVROWS, P), np.float16)
    devpos = np.empty(V, np.int64)
    src = 0
    bset = set(BASES)
    for pos in range(DEVC):
        if pos in bset:
            continue  # zero sentinel
        dev[pos, :D] = t16[src]
        devpos[src] = pos
        src += 1
    assert src == V
    dev[DEVC:] = dev[: DEVROWS - DEVC]
    return dev, devpos


def _feasible_rels(d):
    """[(pass, window-relative idx)] for core position d, incl. wrap copy."""
    out = [(q, d - BASES[q]) for q in range(NQ)
           if BASES[q] <= d < BASES[q] + WIN]
    if d + DEVC < DEVROWS:  # duplicated prefix: reachable from the last pass
        out.append((NQ - 1, d + DEVC - BASES[NQ - 1]))
    return out


def _balance_passes(devrows_sample):
    """Assign each device-row index to a feasible pass, minimizing the max
    per-pass count. Windows overlap adjacently on a circle, so flexible
    indices sit on edges of a cycle -> binary search on T; for each T scan
    the wrap-edge split and run left-greedy water-filling on the path.
    Returns list of NQ lists of window-relative indices."""
    fixed = [[] for _ in range(NQ)]
    flex = [[] for _ in range(NQ)]  # edge e: passes (e, (e+1)%NQ)
    for d in devrows_sample:
        feas = _feasible_rels(d)
        if len(feas) == 1:
            fixed[feas[0][0]].append(feas[0][1])
        else:
            qs = sorted(q for q, _ in feas)
            e = NQ - 1 if qs == [0, NQ - 1] else qs[0]
            flex[e].append(dict(feas))
    f = [len(g) for g in fixed]
    x = [len(e) for e in flex]

    def path_ok(T, t3):
        # t3 wrap items to pass NQ-1; the rest (x[NQ-1]-t3) preload pass 0
        takes = [0] * (NQ - 1)
        carry = x[NQ - 1] - t3
        for q in range(NQ - 1):
            load = f[q] + carry
            if load > T:
                return None
            takes[q] = min(x[q], T - load)
            carry = x[q] - takes[q]
        if f[NQ - 1] + carry + t3 > T:
            return None
        return takes

    lo = max(1, -(-len(devrows_sample) // NQ))
    hi = max(lo, max(f) + sum(x))
    best = None
    while lo < hi:
        mid = (lo + hi) // 2
        sol = next((
            (t3, tk) for t3 in range(x[NQ - 1] + 1)
            if (tk := path_ok(mid, t3)) is not None), None)
        if sol is not None:
            hi = mid
        else:
            lo = mid + 1
    T = lo
    t3, takes = next((t3, tk) for t3 in range(x[NQ - 1] + 1)
                     if (tk := path_ok(T, t3)) is not None)

    groups = [list(fixed[q]) for q in range(NQ)]
    # wrap edge: t3 items to pass NQ-1, rest to pass 0
    for i, item in enumerate(flex[NQ - 1]):
        q = NQ - 1 if i < t3 else 0
        groups[q].append(item[q])
    for e in range(NQ - 1):
        for i, item in enumerate(flex[e]):
            q = e if i < takes[e] else e + 1
            groups[q].append(item[q])
    return groups


def preprocess(table, indices, lengths):
    """Host prep. Returns (in_maps, g_sched, order) where order[r] is the
    original sample id at global dealt rank r."""
    dev, devpos = _dev_table(table)

    idx_np = np.asarray(indices, dtype=np.int64)  # [B, L]
    lens = np.asarray(lengths).astype(np.int64)  # [B]
    inv_len = (1.0 / np.maximum(lens, 1)).astype(np.float32)

    # per-sample pass groups (window-relative indices)
    sample_groups = []
    cnt = np.zeros((B, NQ), np.int64)
    for s in range(B):
        drows = devpos[idx_np[s, : lens[s]]]
        sample_groups.append(_balance_passes(drows))
        cnt[s] = [len(g) for g in sample_groups[s]]

    # greedy deal: assign samples to the 16 global blocks (1024 each) to
    # minimize the per-block per-pass maxima; rank r -> (block r//1024,
    # core (r%1024)//128, partition r%128)
    key = cnt.max(1) * 64 + lens
    pool = np.argsort(-key, kind="stable")
    gmax = np.zeros((NBLK, NQ), np.int64)
    fill = np.zeros(NBLK, np.int64)
    assign = np.empty(B, np.int64)
    for s in pool:
        best, bc = -1, None
        for b in range(NBLK):
            if fill[b] >= 1024:
                continue
            cost = np.maximum(gmax[b], cnt[s]).sum() - gmax[b].sum()
            if bc is None or cost < bc:
                best, bc = b, cost
        assign[s] = best
        gmax[best] = np.maximum(gmax[best], cnt[s])
        fill[best] += 1
    order = np.concatenate([pool[assign[pool] == b] for b in range(NBLK)])

    g_sched = [[int(x) for x in np.maximum(gmax[b], 1)] for b in range(NBLK)]

    wcols = [[g * P // 16 for g in r] for r in g_sched]
    WC = sum(sum(r) for r in wcols)

    in_maps = []
    for c in range(NCORES):
        idx16 = np.zeros((P, WC), np.int16)
        invl_dev = np.empty((P, NBLK), np.float32)
        col = 0
        for b in range(NBLK):
            ranks = order[b * 1024 + c * P : b * 1024 + (c + 1) * P]
            invl_dev[:, b] = inv_len[ranks]
            for q in range(NQ):
                gq = g_sched[b][q]
                blk = np.zeros((P, gq), np.int16)  # sentinel rel idx 0
                for p, s in enumerate(ranks):
                    grp = sample_groups[s][q]
                    blk[p, : len(grp)] = grp
                # stream order i = c*128 + p -> wrap int16 [16, nidx/16] x8
                flat = blk.T.ravel()  # [gq*128]
                w = flat.reshape(gq * P // 16, 16).T  # [16, cols]
                nw = wcols[b][q]
                idx16[:, col : col + nw] = np.tile(w, (8, 1))
                col += nw
        in_maps.append(
            {
                "table": dev,
                "idx": np.ascontiguousarray(idx16),
                "inv_len": np.ascontiguousarray(invl_dev),
            }
        )
    return in_maps, g_sched, order


def kernel(table, indices, lengths):
    in_maps, g_sched, order = preprocess(table, indices, lengths)
    key = tuple(tuple(r) for r in g_sched)
    nc = _CACHE.get(key)
    if nc is None:
        nc = _CACHE[key] = build(g_sched)
    res = bass_utils.run_bass_kernel_spmd(nc, in_maps, core_ids=list(range(NCORES)))
    full = np.empty((B, D), np.float32)
    for b in range(NBLK):
        for c in range(NCORES):
            ranks = order[b * 1024 + c * P : b * 1024 + (c + 1) * P]
            full[ranks] = res.results[c]["out"][b]
    return full


# revision 37
# speedup vs baseline: 2.0308x; 1.2037x over previous
"""EmbeddingBag-mean (padded ragged gather + masked mean) on 8 Trainium2 cores.

Strategy (data-parallel over batch):
  - Each of the 8 cores owns B/8 = 2048 samples; the embedding table is
    replicated to every core's HBM as fp16, rows padded to 128 elements
    (256 B stride) so the MoE `dma_gather` ucode (stride in 256 B units,
    int16 indices) can gather single 128 B rows.
  - int16 indices only reach 32768 rows, so the gather runs as NQ=5 passes
    over overlapping 32768-row windows whose bases (spacing 20001) tile a
    circle: the first WIN-spacing core rows are duplicated after the table
    end, so every pass has the same (small) exclusive index mass.  A zero
    sentinel row sits at each window base (relative index 0).  Flexible
    indices (in two windows' overlap) sit on edges of a cycle; per sample,
    exact min-max water-filling (binary search + wrap-edge scan) equalizes
    per-pass counts to ~ceil(len/5).
  - Samples are dealt greedily to 16 global blocks of 1024 (8 cores x 128
    partitions) minimizing the per-block per-pass slot maxima G[b][q]
    (shared across cores; one SPMD module).  Pad slots -> sentinel row.
  - Device kernel (per core), per block of 128 samples:
      1. NQ dma_gather (queue = issue counter % 4, periodic for Tile's
         DMASW lanes): g[p, off_q + j, :] = window_q[idx16[...], :64]
         (128 B descriptors; manual instruction to bypass the bass-level
         elem_size%256 assert, which the ucode only needs for transpose)
      2. one DVE tensor_reduce over all slot columns (fp16 in, fp32 out)
      3. ACT Copy-with-scale by 1/max(len,1) (per-partition scalar)
      4. DMA the [128, 64] fp32 block out
  - Host un-permutes the global deal and returns [B, 64] fp32.
"""

import numpy as np

try:
    import concourse.bacc as bacc
except ImportError:  # harness containers keep the repo at /opt/trn_rl_repo
    import sys

    sys.path.insert(0, "/opt/trn_rl_repo")
    import concourse.bacc as bacc

import concourse.bass as bass
import concourse.mybir as mybir
import concourse.tile as tile
from concourse import bass_utils

B, L, V, D = 16384, 50, 100000, 64
NCORES = 8
P = 128
BC = B // NCORES  # 2048 samples per core
NBLK = BC // P  # 16 blocks of 128 samples
NQ = 5  # gather passes (overlapping windows)
WIN = 32768  # int16-reachable rows per pass
DEVC = V + NQ  # core device rows: table + one zero sentinel per window

# wrap layout: windows tile a circle (the first WIN-S core rows are
# duplicated after the end), so the pass-exclusive mass is uniform instead
# of piling onto the outer windows.  Sentinel zero row at each window base.
_SPACING = -(-DEVC // NQ)  # ceil
BASES = [q * _SPACING for q in range(NQ)]
DEVROWS = BASES[-1] + WIN  # core + duplicated prefix
assert DEVROWS >= DEVC and _SPACING < WIN

_CACHE: dict = {}


def _manual_dma_gather(nc, out_ap, in_ap, idxs_ap, num_idxs, elem_size,
                       queue_num, single_packet):
    """dma_gather without the elem_size_bytes%256 assert (stride is 256B)."""
    gp = nc.gpsimd
    _in_ap = gp.lower_ap_dma(in_ap, for_custom_bir_dma=True)
    _idxs_ap = gp.lower_ap(idxs_ap)
    _out_ap = gp.lower_ap(out_ap)
    stride_bytes = in_ap.ap[0][0] * mybir.dt.size(in_ap.dtype)
    assert stride_bytes % 256 == 0
    return gp.add_instruction(
        mybir.InstDMAGatherAnt(
            name=nc.get_next_instruction_name(),
            ins=[*_in_ap, _idxs_ap, gp.lower_val_access(gp.to_reg(num_idxs))],
            outs=[_out_ap],
            transpose=False,
            num_idxs=num_idxs,
            elem_size=elem_size,
            stride_bytes_256=stride_bytes // 256,
            gen_mode=0,
            single_packet=single_packet,
            queue_num=queue_num,
            sbuf_tokens_per_rank=0,
            sbuf_free_dim_per_rank=0,
            sbuf_free_dim_pad_per_rank=0,
            sbuf_byte_offset=0,
        )
    )


def build(g_sched, reps: int = 1, mode: str = "full", qpat=None):
    """Build + compile the per-core Bass module.

    g_sched: [NBLK][NQ] gather slot counts (>=1 each).
    reps > 1 wraps the block loop in tc.For_i for slope timing.
    mode: "full" | "gather" (skip reduce/scale/out) | "nored" (skip reduce).
    """
    g_sched = [list(r) for r in g_sched]
    assert len(g_sched) == NBLK and all(len(r) == NQ for r in g_sched)
    gtot = [sum(r) for r in g_sched]
    g_max = max(gtot)
    # idx16 column layout: per (block, pass) a run of G*P/16 int16 columns
    wcols = [[g * P // 16 for g in r] for r in g_sched]
    WC = sum(sum(r) for r in wcols)

    nc = bacc.Bacc("TRN2", target_bir_lowering=False, debug=False,
                   num_swdge_queues=4)
    table = nc.dram_tensor("table", [DEVROWS, P], mybir.dt.float16,
                           kind="ExternalInput")
    idx = nc.dram_tensor("idx", [P, WC], mybir.dt.int16, kind="ExternalInput")
    inv_len = nc.dram_tensor("inv_len", [P, NBLK], mybir.dt.float32,
                             kind="ExternalInput")
    out = nc.dram_tensor("out", [NBLK, P, D], mybir.dt.float32,
                         kind="ExternalOutput")

    with tile.TileContext(nc) as tc:
        with (
            tc.tile_pool(name="const", bufs=1) as cpool,
            tc.tile_pool(name="gather", bufs=4) as gpool,
            tc.tile_pool(name="res", bufs=4) as rpool,
        ):
            idx_sb = cpool.tile([P, WC], mybir.dt.int16)
            nc.sync.dma_start(idx_sb[:], idx.ap())
            invl_sb = cpool.tile([P, NBLK], mybir.dt.float32)
            nc.sync.dma_start(invl_sb[:], inv_len.ap())

            # one gather per (block, pass); queue follows a periodic-8
            # pattern (Tile's DMASW lanes lock to the queue of their first
            # user, so the sequence must repeat with period dividing 8).
            # Pick the multiset-{0,0,1,1,2,2,3,3} pattern that best
            # balances this schedule's instruction sizes across queues.
            import itertools

            sizes = [g_sched[b][q] for b in range(NBLK) for q in range(NQ)]
            best_pat, best_max = tuple([0, 1, 2, 3] * 2), float("inf")
            for pat in set(itertools.permutations([0, 0, 1, 1, 2, 2, 3, 3])):
                loads = [0] * 4
                for i, s in enumerate(sizes):
                    loads[pat[i % 8]] += s
                if max(loads) < best_max:
                    best_pat, best_max = pat, max(loads)
            ictr = [0]

            def body():
                col = 0
                for b in range(NBLK):
                    g = gpool.tile([P, g_max, D], mybir.dt.float16, tag="g")
                    off = 0
                    for q in range(NQ):
                        gq = g_sched[b][q]
                        win = table.ap()[BASES[q] : BASES[q] + WIN, :D]
                        _manual_dma_gather(
                            nc,
                            g[:, off : off + gq, :],
                            win,
                            idx_sb[:, col : col + wcols[b][q]],
                            gq * P,
                            D,
                            queue_num=best_pat[ictr[0] % 8],
                            single_packet=(gq <= 8),
                        )
                        ictr[0] += 1
                        off += gq
                        col += wcols[b][q]
                    if mode == "gather":
                        continue
                    red = rpool.tile([P, D], mybir.dt.float32, tag="red")
                    if mode == "nored":
                        nc.vector.tensor_copy(red[:], g[:, 0, :])
                    else:
                        nc.vector.tensor_reduce(
                            out=red[:],
                            in_=g[:, : gtot[b], :].rearrange("p l d -> p d l"),
                            axis=mybir.AxisListType.X,
                            op=mybir.AluOpType.add,
                        )
                    o = rpool.tile([P, D], mybir.dt.float32, tag="o")
                    nc.scalar.activation(
                        o[:],
                        red[:],
                        mybir.ActivationFunctionType.Copy,
                        scale=invl_sb[:, b : b + 1],
                    )
                    nc.sync.dma_start(out.ap()[b], o[:])

            if reps == 1:
                body()
            else:
                with tc.For_i(0, reps, 1):
                    body()

    nc.compile()
    return nc


def _dev_table(table):
    """fp16 device table [DEVROWS, 128]: zero sentinel at each window base,
    original row r at core position devpos[r], first DEVROWS-DEVC core rows
    duplicated after the end (wrap)."""
    t16 = np.asarray(table, dtype=np.float32).astype(np.float16)
    dev = np.zeros((DEVROWS, P), np.float16)
    devpos = np.empty(V, np.int64)
    src = 0
    bset = set(BASES)
    for pos in range(DEVC):
        if pos in bset:
            continue  # zero sentinel
        dev[pos, :D] = t16[src]
        devpos[src] = pos
        src += 1
    assert src == V
    dev[DEVC:] = dev[: DEVROWS - DEVC]
    return dev, devpos


def _feasible_rels(d):
    """[(pass, window-relative idx)] for core position d, incl. wrap copy."""
    out = [(q, d - BASES[q]) for q in range(NQ)
           if BASES[q] <= d < BASES[q] + WIN]
    if d + DEVC < DEVROWS:  # duplicated prefix: reachable from the last pass
        out.append((NQ - 1, d + DEVC - BASES[NQ - 1]))
    return out


def _balance_passes(devrows_sample):
    """Assign each device-row index to a feasible pass, minimizing the max
    per-pass count. Windows overlap adjacently on a circle, so flexible
    indices sit on edges of a cycle -> binary search on T; for each T scan
    the wrap-edge split and run left-greedy water-filling on the path.
    Returns list of NQ lists of window-relative indices."""
    fixed = [[] for _ in range(NQ)]
    flex = [[] for _ in range(NQ)]  # edge e: passes (e, (e+1)%NQ)
    for d in devrows_sample:
        feas = _feasible_rels(d)
        if len(feas) == 1:
            fixed[feas[0][0]].append(feas[0][1])
        else:
            qs = sorted(q for q, _ in feas)
            e = NQ - 1 if qs == [0, NQ - 1] else qs[0]
            flex[e].append(dict(feas))
    f = [len(g) for g in fixed]
    x = [len(e) for e in flex]

    def path_ok(T, t3):
        # t3 wrap items to pass NQ-1; the rest (x[NQ-1]-t3) preload pass 0
        takes = [0] * (NQ - 1)
        carry = x[NQ - 1] - t3
        for q in range(NQ - 1):
            load = f[q] + carry
            if load > T:
                return None
            takes[q] = min(x[q], T - load)
            carry = x[q] - takes[q]
        if f[NQ - 1] + carry + t3 > T:
            return None
        return takes

    lo = max(1, -(-len(devrows_sample) // NQ))
    hi = max(lo, max(f) + sum(x))
    best = None
    while lo < hi:
        mid = (lo + hi) // 2
        sol = next((
            (t3, tk) for t3 in range(x[NQ - 1] + 1)
            if (tk := path_ok(mid, t3)) is not None), None)
        if sol is not None:
            hi = mid
        else:
            lo = mid + 1
    T = lo
    t3, takes = next((t3, tk) for t3 in range(x[NQ - 1] + 1)
                     if (tk := path_ok(T, t3)) is not None)

    groups = [list(fixed[q]) for q in range(NQ)]
    # wrap edge: t3 items to pass NQ-1, rest to pass 0
    for i, item in enumerate(flex[NQ - 1]):
        q = NQ - 1 if i < t3 else 0
        groups[q].append(item[q])
    for e in range(NQ - 1):
        for i, item in enumerate(flex[e]):
            q = e if i < takes[e] else e + 1
            groups[q].append(item[q])
    return groups


def preprocess(table, indices, lengths):
    """Host prep. Returns (in_maps, g_sched, order) where order[r] is the
    original sample id at global dealt rank r."""
    dev, devpos = _dev_table(table)

    idx_np = np.asarray(indices, dtype=np.int64)  # [B, L]
    lens = np.asarray(lengths).astype(np.int64)  # [B]
    inv_len = (1.0 / np.maximum(lens, 1)).astype(np.float32)

    # per-sample pass groups (window-relative indices)
    sample_groups = []
    cnt = np.zeros((B, NQ), np.int64)
    for s in range(B):
        drows = devpos[idx_np[s, : lens[s]]]
        sample_groups.append(_balance_passes(drows))
        cnt[s] = [len(g) for g in sample_groups[s]]

    # greedy deal: assign samples to the 16 global blocks (1024 each) to
    # minimize the per-block per-pass maxima; rank r -> (block r//1024,
    # core (r%1024)//128, partition r%128)
    key = cnt.max(1) * 64 + lens
    pool = np.argsort(-key, kind="stable")
    gmax = np.zeros((NBLK, NQ), np.int64)
    fill = np.zeros(NBLK, np.int64)
    assign = np.empty(B, np.int64)
    for s in pool:
        best, bc = -1, None
        for b in range(NBLK):
            if fill[b] >= 1024:
                continue
            cost = np.maximum(gmax[b], cnt[s]).sum() - gmax[b].sum()
            if bc is None or cost < bc:
                best, bc = b, cost
        assign[s] = best
        gmax[best] = np.maximum(gmax[best], cnt[s])
        fill[best] += 1
    order = np.concatenate([pool[assign[pool] == b] for b in range(NBLK)])

    g_sched = [[int(x) for x in np.maximum(gmax[b], 1)] for b in range(NBLK)]

    wcols = [[g * P // 16 for g in r] for r in g_sched]
    WC = sum(sum(r) for r in wcols)

    in_maps = []
    for c in range(NCORES):
        idx16 = np.zeros((P, WC), np.int16)
        invl_dev = np.empty((P, NBLK), np.float32)
        col = 0
        for b in range(NBLK):
            ranks = order[b * 1024 + c * P : b * 1024 + (c + 1) * P]
            invl_dev[:, b] = inv_len[ranks]
            for q in range(NQ):
                gq = g_sched[b][q]
                blk = np.zeros((P, gq), np.int16)  # sentinel rel idx 0
                for p, s in enumerate(ranks):
                    grp = sample_groups[s][q]
                    blk[p, : len(grp)] = grp
                # stream order i = c*128 + p -> wrap int16 [16, nidx/16] x8
                flat = blk.T.ravel()  # [gq*128]
                w = flat.reshape(gq * P // 16, 16).T  # [16, cols]
                nw = wcols[b][q]
                idx16[:, col : col + nw] = np.tile(w, (8, 1))
                col += nw
        in_maps.append(
            {
                "table": dev,
                "idx": np.ascontiguousarray(idx16),
                "inv_len": np.ascontiguousarray(invl_dev),
            }
        )
    return in_maps, g_sched, order


def kernel(table, indices, lengths):
    in_maps, g_sched, order = preprocess(table, indices, lengths)
    key = tuple(tuple(r) for r in g_sched)
    nc = _CACHE.get(key)
    if nc is None:
        nc = _CACHE[key] = build(g_sched)
    res = bass_utils.run_bass_kernel_spmd(nc, in_maps, core_ids=list(range(NCORES)))
    full = np.empty((B, D), np.float32)
    for b in range(NBLK):
        for c in range(NCORES):
            ranks = order[b * 1024 + c * P : b * 1024 + (c + 1) * P]
            full[ranks] = res.results[c]["out"][b]
    return full
